# revision 28
# baseline (speedup 1.0000x reference)
"""Causal GQA self-attention with RoPE for TRN2, 8 NeuronCores.

Problem: B=2, S=2048, D=2048, H=16 q-heads, KV=4 kv-heads, HD=128.

Sharding: core c = (batch b = c//4, kv-group g = c%4). Each core computes
q-heads 4g..4g+3 and kv-head g for batch b in the transposed (S^T) layout,
then a partial output projection; host sums the 4 partials per batch.

v6 notes (on top of v4's fused PE-saturated design):
  - startup: x chunk 0 leads the sync DMA ring, wk/wv are halved and the
    first x chunks split so the first K/V matmul has data ~4us earlier;
    cos/sin/wq/masks/wo are dispatched strictly after the x stream they
    used to compete with.
  - PE warm-up: a few zero matmuls on a memset tile run during the DMA
    wait so the HAM clock-gate (1.2->2.4GHz after ~3.4us busy) lifts
    before the real stream begins.
  - the ops accumulator, softmax-denominator psum and output-projection
    psum share one 4-slot PSUM ring: a head's first PV no longer waits
    on the previous head's attnT write (was ~0.7us/head).
  - diagonal exp tiles are narrowed to the live query range (ACT slack).
  - drained output-projection matmuls are emitted BEFORE each pair's PV
    (the PE queue is strict FIFO: behind PV they could not fill the
    exp-latency hole at each head start, ~1us/head); the per-head tail
    drains likewise moved ahead of the colsum/recip chain.
  - job-drain cadence slowed (odd pairs only for jq2/jq3) so output-
    projection fill lasts into the late heads; the final 16 jobs store
    2 chunks per DMA descriptor.
"""
import sys

sys.path.insert(0, "/opt/trn_rl_repo")

import numpy as np

import concourse.tile as tile
from concourse import bacc, mybir
from concourse.bass_utils import run_bass_kernel_spmd

F32 = mybir.dt.float32
F16 = mybir.dt.float16
AF = mybir.ActivationFunctionType
OP = mybir.AluOpType

P = 128          # partitions / head dim
S = 2048         # sequence length
D = 2048         # model dim
NH = 4           # q heads per core
QW = NH * P      # q projection width per core (512)
NKD = D // P     # contraction chunks (16)
QCH = 512        # query chunk (free dim of attention matmuls)
NQC = S // QCH   # 4
KCH = P          # key chunk (128, on partitions)
NKC = S // KCH   # 16
NOC = D // P     # output-projection chunks (16)
SCALE = float(P) ** -0.5


def _host_constants():
    inv = 1.0 / (10000.0 ** (np.arange(0, P, 2, dtype=np.float64) / P))  # [64]
    pos = np.arange(S, dtype=np.float64)
    freqs = pos[:, None] * inv[None, :]                  # [S, 64]
    emb = np.concatenate([freqs, freqs], axis=-1)        # [S, 128]
    cosT = np.cos(emb).T.astype(np.float16).copy()       # [128, S]
    sinT = np.sin(emb).T.astype(np.float16)
    sinT[: P // 2] *= np.float16(-1.0)                   # fold rotate_half sign
    sinT = sinT.copy()
    # triangular mask for the 128-wide diagonal sliver: m[p, q] = q >= p
    tri = (np.arange(P)[None, :] >= np.arange(P)[:, None]).astype(np.float16)
    # full step masks for the jq0/h0 full-width path: mask[p,j,q] = q >= p+128j
    q = np.arange(QCH)[None, None, :]
    p = np.arange(P)[:, None, None]
    j = np.arange(4)[None, :, None]
    masks = (q >= p + KCH * j).astype(np.float16)        # [128, 4, 512]
    ones = np.ones((P, P), dtype=np.float16)
    # rotate-half permutation: out[m] = in[(m+64) % 128]
    perm = np.zeros((P, P), dtype=np.float16)
    perm[(np.arange(P) + P // 2) % P, np.arange(P)] = 1.0
    return cosT, sinT, tri, masks, ones, perm


def build_nc():
    cosT_np, sinT_np, tri_np, masks_np, ones_np, perm_np = _host_constants()

    nc = bacc.Bacc(None)
    # weights arrive pre-packed as [128, ...] partition-major arrays
    xT_d = nc.dram_tensor("xT", [D, S], F16, kind="ExternalInput")
    wq_d = nc.dram_tensor("wq", [P, NKD * QW], F16, kind="ExternalInput")
    wk_d = nc.dram_tensor("wk", [P, NKD * P], F16, kind="ExternalInput")
    wv_d = nc.dram_tensor("wv", [P, NKD * P], F16, kind="ExternalInput")
    wo_d = nc.dram_tensor("wo", [P, NH * D], F16, kind="ExternalInput")
    out_d = nc.dram_tensor("outT", [D, S], F16, kind="ExternalOutput")

    cos_d = nc.inline_tensor(cosT_np, name="cosT")
    sin_d = nc.inline_tensor(sinT_np, name="sinT")
    perm_d = nc.inline_tensor(perm_np, name="permm")
    del tri_np, masks_np, ones_np  # generated on-device (DMA diet)

    xT_v = xT_d[:].rearrange("(kd p) s -> p kd s", p=P)
    out_v = out_d[:].rearrange("(oc p) s -> p oc s", p=P)

    # alternate DMA dispatch between the sync and gpsimd queues
    dq = [0]

    def dma(out, in_):
        eng = nc.sync if dq[0] % 2 == 0 else nc.gpsimd
        dq[0] += 1
        eng.dma_start(out, in_)

    with tile.TileContext(nc) as tc:
        with tc.tile_pool(name="persist", bufs=1) as pp:
            qT = pp.tile([P, NH, S], F16)        # q^T; attention overwrites
            kT = pp.tile([P, S], F16)
            vT = pp.tile([P, S], F16)
            vK = pp.tile([P, NKC, P], F16)       # V as (kpos, kchunk, hd)
            mask_t = pp.tile([P, 2, QCH], F16)   # step mask, on-device gen
            ones_t = pp.tile([P, P], F16)        # all-ones for colsum
            perm_t = pp.tile([P, P], F16)
            # triangular [P,P] sliver mask is the j=0 prefix of mask_t
            tri_t = mask_t[:, 0, 0:P]

            # attention output overwrites qT in place: slice (h, jq-chunk) is
            # written only after every read of that same slice is done.
            attnT = qT

            # ======== Phase 1: QKV projections + RoPE ====================
            # xp/wp/p1/p1c stay open through the fused phase: the q3 head
            # projection is deferred into the first attention block
            # (which otherwise idles PE waiting on exp latency)
            _outer = [
                tc.tile_pool(name="xp", bufs=1),
                tc.tile_pool(name="wp", bufs=1),
                tc.tile_pool(name="p1", bufs=2),
                tc.tile_pool(name="p1c", bufs=1),
            ]
            xp = _outer[0].__enter__()
            wp = _outer[1].__enter__()
            p1 = _outer[2].__enter__()
            p1c = _outer[3].__enter__()
            with tc.tile_pool(name="psP", bufs=1, space="PSUM") as psP:
                wkt = wp.tile([P, NKD, P], F16)
                wvt = wp.tile([P, NKD, P], F16)
                wqt = wp.tile([P, NKD, QW], F16)
                xf = xp.tile([P, NKD, S], F16)
                cos_t = p1c.tile([P, S], F16)
                sin_t = p1c.tile([P, S], F16)
                wk_v = wk_d[:].rearrange("p (kd c) -> p kd c", c=P)
                wv_v = wv_d[:].rearrange("p (kd c) -> p kd c", c=P)
                wq_vv = wq_d[:].rearrange("p (kd c) -> p kd c", c=QW)
                H8 = NKD // 2

                # PE warm-up: lift the HAM clock gate during the x DMA wait
                warm_w = p1c.tile([P, P], F16)
                warm_x = p1c.tile([P, QCH], F16)
                nc.vector.memset(warm_w[:], 0.0)
                nc.vector.memset(warm_x[:], 0.0)
                warm_ps = psP.tile([P, QCH], F32, tag="pk0", name="pk0")
                for _ in range(8):
                    nc.tensor.matmul(warm_ps[:], warm_w[:], warm_x[:],
                                     start=True, stop=True)

                # dispatch order == need order; the x stream round-robins
                # over FOUR queues (each DMA ring tops out well below the
                # per-core HBM budget, so more rings saturate sooner);
                # weights/constants follow strictly behind the x stream
                sync_loads = [
                    (xf[:, 0, :], xT_v[:, 0, :]),
                    (xf[:, 3, :], xT_v[:, 3, :]),
                    (xf[:, 6, :], xT_v[:, 6, :]),
                    (xf[:, 9, :], xT_v[:, 9, :]),
                    (xf[:, 12, :], xT_v[:, 12, :]),
                    (cos_t[:], cos_d[:]),
                    (wqt[:, 0:4, :], wq_vv[:, 0:4, :]),
                    (wqt[:, 8:12, :], wq_vv[:, 8:12, :]),
                ]
                gp_loads = [
                    (wkt[:, :H8, :], wk_v[:, :H8, :]),
                    (wvt[:, :H8, :], wv_v[:, :H8, :]),
                    (xf[:, 2, :], xT_v[:, 2, :]),
                    (wkt[:, H8:, :], wk_v[:, H8:, :]),
                    (wvt[:, H8:, :], wv_v[:, H8:, :]),
                    (xf[:, 5, :], xT_v[:, 5, :]),
                    (xf[:, 8, :], xT_v[:, 8, :]),
                    (xf[:, 11, :], xT_v[:, 11, :]),
                    (xf[:, 14, :], xT_v[:, 14, :]),
                    (sin_t[:], sin_d[:]),
                    (wqt[:, 4:8, :], wq_vv[:, 4:8, :]),
                    (wqt[:, 12:16, :], wq_vv[:, 12:16, :]),
                ]
                sc_loads = [
                    (xf[:, 1, :], xT_v[:, 1, :]),
                    (xf[:, 4, :], xT_v[:, 4, :]),
                    (xf[:, 7, :], xT_v[:, 7, :]),
                    (xf[:, 10, :], xT_v[:, 10, :]),
                    (xf[:, 13, :], xT_v[:, 13, :]),
                    (xf[:, 15, :], xT_v[:, 15, :]),
                    (perm_t[:], perm_d[:]),
                ]
                for o, i_ in sync_loads:
                    nc.sync.dma_start(o, i_)
                for o, i_ in gp_loads:
                    nc.gpsimd.dma_start(o, i_)
                for o, i_ in sc_loads:
                    nc.scalar.dma_start(o, i_)

                # on-device constants (DMA diet): the causal step mask
                # (tri is its j=0 prefix) and the all-ones colsum matrix
                nc.gpsimd.memset(ones_t[:], 1.0)
                nc.gpsimd.memset(mask_t[:], 1.0)
                nc.gpsimd.affine_select(
                    mask_t[:], mask_t[:],
                    pattern=[[-KCH, 2], [1, QCH]],
                    compare_op=OP.is_ge, fill=0.0,
                    base=0, channel_multiplier=-1,
                )

                def rope_chunk(dst, rot_ps, cslice):
                    # dst = dst*cos + perm(dst)*sin ; rot_ps holds perm(dst)
                    tmpv = p1.tile([P, QCH], F16, tag="ropet", bufs=3)
                    nc.vector.tensor_tensor(
                        tmpv[:], rot_ps[:], sin_t[:, cslice], OP.mult)
                    nc.vector.tensor_tensor(
                        dst, dst, cos_t[:, cslice], OP.mult)
                    nc.vector.tensor_tensor(dst, dst, tmpv[:], OP.add)

                # K and V projections, kd-outer, 8 PSUM accumulators
                psK = [psP.tile([P, QCH], F32, tag=f"pk{jr}", name=f"pk{jr}")
                       for jr in range(NQC)]
                psV = [psP.tile([P, QCH], F32, tag=f"pv{jr}", name=f"pv{jr}")
                       for jr in range(NQC)]
                for kd in range(NKD):
                    for jr in range(NQC):
                        nc.tensor.matmul(
                            psK[jr][:], wkt[:, kd, :],
                            xf[:, kd, jr * QCH : (jr + 1) * QCH],
                            start=(kd == 0), stop=(kd == NKD - 1),
                        )
                    for jr in range(NQC):
                        nc.tensor.matmul(
                            psV[jr][:], wvt[:, kd, :],
                            xf[:, kd, jr * QCH : (jr + 1) * QCH],
                            start=(kd == 0), stop=(kd == NKD - 1),
                        )
                for jr in range(NQC):
                    nc.scalar.copy(
                        out=kT[:, jr * QCH : (jr + 1) * QCH], in_=psK[jr][:]
                    )
                # pre-warm the exp table set while ACT is idle-ish
                warm = p1.tile([P, 1], F32, tag="warm")
                nc.scalar.activation(warm[:], psK[0][:, 0:1], AF.Exp, scale=1.0)
                for jr in range(NQC):
                    nc.scalar.copy(
                        out=vT[:, jr * QCH : (jr + 1) * QCH], in_=psV[jr][:]
                    )
                # rope kT chunk by chunk (perm matmul reuses freed V banks)
                for jr in range(NQC):
                    cs = slice(jr * QCH, (jr + 1) * QCH)
                    rot = psP.tile([P, QCH], F32, tag=f"pv{jr}",
                                   name=f"pv{jr}")
                    nc.tensor.matmul(rot[:], perm_t[:], kT[:, cs],
                                     start=True, stop=True)
                    rope_chunk(kT[:, cs], rot, cs)
                nc.sync.dma_start_transpose(vK[:], vT[:])

                # Q projections per head, ping-pong PSUM; each chunk's
                # rot/rope is deferred by one chunk so the rot matmul
                # (which waits on the ACT copy) never blocks the PE FIFO
                pending_rope = [None]

                def flush_rope(idx):
                    if pending_rope[0] is None:
                        return
                    pdst, pcs = pending_rope[0]
                    pending_rope[0] = None
                    rot = psP.tile([P, QCH], F32, tag=f"pk{2 + idx % 2}",
                                   name=f"pk{2 + idx % 2}")
                    nc.tensor.matmul(rot[:], perm_t[:], pdst,
                                     start=True, stop=True)
                    rope_chunk(pdst, rot, pcs)

                ci = 0
                for hh in range(NH - 1):
                    for jr in range(NQC):
                        cs = slice(jr * QCH, (jr + 1) * QCH)
                        ps = psP.tile([P, QCH], F32, tag=f"pk{jr % 2}",
                                      name=f"pk{jr % 2}")
                        for kd in range(NKD):
                            nc.tensor.matmul(
                                ps[:],
                                wqt[:, kd, hh * P : (hh + 1) * P],
                                xf[:, kd, jr * QCH : (jr + 1) * QCH],
                                start=(kd == 0), stop=(kd == NKD - 1),
                            )
                        dst = qT[:, hh, cs]
                        nc.scalar.copy(out=dst, in_=ps[:])
                        flush_rope(ci)
                        pending_rope[0] = (dst, cs)
                        ci += 1
                flush_rope(ci)

            # wo prefetched during phase 1 tail / attention start
            p3w_cm = tc.tile_pool(name="p3w", bufs=1)
            p3w = p3w_cm.__enter__()
            wo_t = p3w.tile([P, NH, D], F16)
            wo_vv = wo_d[:].rearrange("p (a o) -> p a o", a=NH)
            dma(wo_t[:, 0:2, :], wo_vv[:, 0:2, :])
            dma(wo_t[:, 2:4, :], wo_vv[:, 2:4, :])

            # ======== Phase 2: fused attention + output projection =======
            # PSUM budget (8 banks): s0,s1 = 2+2; ops/dps/po share a 4-slot
            # ring (so a head's ops never waits on the previous head's
            # attnT write, and the output-projection pipeline is 3 deep).
            with tc.tile_pool(name="p2", bufs=1) as p2, \
                 tc.tile_pool(name="psF", bufs=1, space="PSUM") as psF:

                def ring():
                    return psF.tile([P, QCH], F32, tag="po", bufs=4,
                                    name="po")

                # pending output-projection emissions, drained into the
                # exp-latency gaps of the attention stream
                jobs = []

                # deferred q3 projection: 4-matmul groups drained into
                # jq1's h0/h1 exp-latency gaps via the shared ring;
                # q3 is first read at jq1's h3
                q3slot = [None]

                def make_qgroup(jr, g):
                    def qg():
                        cs = slice(jr * QCH, (jr + 1) * QCH)
                        dst = qT[:, 3, cs]
                        if g == 4:
                            # separate drain step: the rot matmul waits on
                            # the g3 ACT copy, so it must not ride the PE
                            # FIFO right behind it
                            rot = ring()
                            nc.tensor.matmul(rot[:], perm_t[:], dst,
                                             start=True, stop=True)
                            rope_chunk(dst, rot, cs)
                            return
                        if g == 0:
                            q3slot[0] = ring()
                        ps = q3slot[0]
                        for kd in range(4 * g, 4 * g + 4):
                            nc.tensor.matmul(
                                ps[:],
                                wqt[:, kd, 3 * P : 4 * P],
                                xf[:, kd, jr * QCH : (jr + 1) * QCH],
                                start=(kd == 0), stop=(kd == NKD - 1),
                            )
                        if g == 3:
                            nc.scalar.copy(out=dst, in_=ps[:])
                    return qg

                qjobs = [make_qgroup(jr, g)
                         for jr in range(NQC) for g in range(5)]

                def qdrain(n):
                    for _ in range(n):
                        if qjobs:
                            qjobs.pop(0)()

                quota = [1 << 30]

                def drain(n):
                    for _ in range(n):
                        if not jobs or quota[0] <= 0:
                            return
                        quota[0] -= 1
                        jobs.pop(0)()

                def make_job(oc, jq):
                    # full 4-head output projection chunk (jq0/jq2)
                    def job():
                        po = ring()
                        for a in range(NH):
                            nc.tensor.matmul(
                                po[:],
                                wo_t[:, a, oc * P : (oc + 1) * P],
                                attnT[:, a, jq * QCH : (jq + 1) * QCH],
                                start=(a == 0), stop=(a == NH - 1),
                            )
                        ot = p2.tile([P, QCH], F16, tag="ot", bufs=4)
                        if oc % 2 == 0:
                            nc.scalar.copy(out=ot[:], in_=po[:])
                        else:
                            nc.vector.tensor_copy(out=ot[:], in_=po[:])
                        dma(out_d[oc * P : (oc + 1) * P,
                                  jq * QCH : (jq + 1) * QCH], ot[:])
                    return job

                def make_final_pair(oc0, jq):
                    # two full chunks sharing one store descriptor (the
                    # final drain, where store-dispatch backlog matters)
                    def job():
                        ot2 = p2.tile([P, 2, QCH], F16, tag="ot2", bufs=3,
                                      name="ot2")
                        for k in range(2):
                            oc = oc0 + k
                            po = ring()
                            for a in range(NH):
                                nc.tensor.matmul(
                                    po[:],
                                    wo_t[:, a, oc * P : (oc + 1) * P],
                                    attnT[:, a, jq * QCH : (jq + 1) * QCH],
                                    start=(a == 0), stop=(a == NH - 1),
                                )
                            if k == 0:
                                nc.scalar.copy(out=ot2[:, k, :], in_=po[:])
                            else:
                                nc.vector.tensor_copy(out=ot2[:, k, :],
                                                      in_=po[:])
                        dma(out_v[:, oc0 : oc0 + 2,
                                  jq * QCH : (jq + 1) * QCH], ot2[:])
                    return job

                # jq1 first: its leading pairs are off-diagonal, so the
                # s tiles' first-ever writes are full-width (no stale
                # columns ever feed downstream), and by the time the
                # tiny DVE-bound jq0 block runs there are output-
                # projection jobs available to keep PE busy.
                for jq in (1, 0, 2, 3):
                    for h in range(NH):
                        nkc = 4 * (jq + 1)
                        npair = nkc // 2
                        qs = qT[:, h, jq * QCH : (jq + 1) * QCH]
                        if jq == 1 and h == 2:
                            # q3 must be fully projected+roped before h3
                            qdrain(len(qjobs))
                        # ration the job fill evenly over this jq's heads
                        quota[0] = max(1, -(-len(jobs) // (NH - h)))
                        ops = ring()
                        pAcc = p2.tile([P, 2 * QCH], F16,
                                       tag="pAcc", bufs=2, name="pAcc")

                        # query offset of the live range for key chunk kc
                        # (0 off the diagonal)
                        def qoff(kc):
                            return max(0, KCH * (kc - 4 * jq))

                        def emit_qk(ip):
                            kc0 = 2 * ip
                            sps = psF.tile(
                                [P, 2 * QCH], F32, tag=f"s{ip % 2}",
                                name=f"sps{ip % 2}",
                            )
                            for k2 in range(2):
                                off = qoff(kc0 + k2)
                                nc.tensor.matmul(
                                    sps[:, k2 * QCH + off : (k2 + 1) * QCH],
                                    kT[:, (kc0 + k2) * P : (kc0 + k2 + 1) * P],
                                    qs[:, off:],
                                    start=True,
                                    stop=True,
                                )
                            return sps

                        sps_cur = emit_qk(0)
                        for ip in range(npair):
                            kc0 = 2 * ip
                            pT = p2.tile([P, 2 * QCH], F16, tag="pT", bufs=6)
                            diag = kc0 >= 4 * jq
                            first_pair_init = ip == 0
                            if (diag and not first_pair_init
                                    and qoff(kc0 + 1) > 0):
                                # narrow exp to the live query range
                                for k2 in range(2):
                                    off = qoff(kc0 + k2)
                                    sl = slice(k2 * QCH + off,
                                               (k2 + 1) * QCH)
                                    nc.scalar.activation(
                                        pT[:, sl], sps_cur[:, sl],
                                        AF.Exp, scale=SCALE
                                    )
                            else:
                                nc.scalar.activation(
                                    pT[:], sps_cur[:], AF.Exp, scale=SCALE
                                )
                            if diag and first_pair_init:
                                # jq0 pair0: full-width mask (also zeroes
                                # dead/stale columns) since the masked tile
                                # is about to initialize pAcc via a full-
                                # width copy
                                nc.vector.tensor_tensor(
                                    pT[:], pT[:],
                                    mask_t[:, kc0 : kc0 + 2, :], OP.mult,
                                )
                            elif diag:
                                # mask only the 128-wide diagonal slivers
                                for k2 in range(2):
                                    off = qoff(kc0 + k2)
                                    sl = slice(k2 * QCH + off,
                                               k2 * QCH + off + KCH)
                                    nc.vector.tensor_tensor(
                                        pT[:, sl], pT[:, sl], tri_t[:],
                                        OP.mult,
                                    )
                            if ip + 1 < npair:
                                sps_cur = emit_qk(ip + 1)
                            # fill the exp-latency hole BEFORE the PV that
                            # waits on it (PE queue is strict FIFO)
                            if jq == 1 and qjobs:
                                qdrain(1)
                            elif jq == 0:
                                drain(2)
                            elif ip == 0 or ip % 2 == 1:
                                # jq2/jq3: pair 0 (the head-start exp hole)
                                # and odd pairs; the per-head quota spreads
                                # the 16 jobs over all four heads
                                drain(1)
                            for k2 in range(2):
                                kc = kc0 + k2
                                off = qoff(kc)
                                nc.tensor.matmul(
                                    ops[:, off:],
                                    vK[:, kc, :],
                                    pT[:, k2 * QCH + off : (k2 + 1) * QCH],
                                    start=(kc == 0),
                                    stop=(kc == nkc - 1),
                                )
                            if first_pair_init:
                                nc.vector.tensor_copy(out=pAcc[:], in_=pT[:])
                            elif qoff(kc0) == 0 and qoff(kc0 + 1) == 0:
                                nc.vector.tensor_tensor(
                                    pAcc[:], pAcc[:], pT[:], OP.add
                                )
                            else:
                                for k2 in range(2):
                                    off = qoff(kc0 + k2)
                                    sl = slice(k2 * QCH + off, (k2 + 1) * QCH)
                                    nc.vector.tensor_tensor(
                                        pAcc[:, sl], pAcc[:, sl], pT[:, sl],
                                        OP.add,
                                    )
                        def tail(ops=ops, pAcc=pAcc, h=h, jq=jq):
                            # PE runway first: the colsum waits on the DVE
                            # pAcc chain anyway, and the next head's QK can
                            # run during the recip/mult latency
                            if jobs:
                                drain(3)
                            else:
                                qdrain(3)
                            # cross-partition colsum of pAcc -> denominator
                            dps = ring()
                            if jq >= 2:
                                # DVE has slack here: pre-add the halves
                                # so one colsum matmul suffices
                                pAccH = p2.tile([P, QCH], F16, tag="pAccH",
                                                bufs=2, name="pAccH")
                                nc.vector.tensor_tensor(
                                    pAccH[:], pAcc[:, 0:QCH],
                                    pAcc[:, QCH : 2 * QCH], OP.add,
                                )
                                nc.tensor.matmul(
                                    dps[:], ones_t[:], pAccH[:],
                                    start=True, stop=True,
                                )
                            else:
                                nc.tensor.matmul(
                                    dps[:], ones_t[:], pAcc[:, 0:QCH],
                                    start=True, stop=False,
                                )
                                nc.tensor.matmul(
                                    dps[:], ones_t[:],
                                    pAcc[:, QCH : 2 * QCH],
                                    start=False, stop=True,
                                )
                            dib = p2.tile([P, QCH], F32, tag="dib", bufs=2,
                                          name="dib")
                            nc.vector.reciprocal_approx_fast(dib[:], dps[:])
                            nc.vector.tensor_tensor(
                                attnT[:, h, jq * QCH : (jq + 1) * QCH],
                                ops[:],
                                dib[:],
                                OP.mult,
                            )

                        tail()
                    if jq == 3:
                        for oc0 in range(0, NOC, 2):
                            jobs.append(make_final_pair(oc0, jq))
                    else:
                        for oc in range(NOC):
                            jobs.append(make_job(oc, jq))
                quota[0] = 1 << 30
                drain(len(jobs))
            p3w_cm.__exit__(None, None, None)
            for _cm in reversed(_outer):
                _cm.__exit__(None, None, None)

    nc.finalize()
    return nc


_NC = None


def _get_nc():
    global _NC
    if _NC is None:
        _NC = build_nc()
    return _NC


def _pack_pm(w):
    """[K, C] f32 -> [128, (K//128)*C] f16 partition-major pack:
    out[p, kd*C + c] = w[kd*128 + p, c]"""
    K, C = w.shape
    kd = K // P
    return np.ascontiguousarray(
        np.asarray(w, dtype=np.float16).reshape(kd, P, C).transpose(1, 0, 2)
    ).reshape(P, kd * C)


def make_in_maps(x, wq, wk, wv, wo):
    x = np.asarray(x, dtype=np.float32)
    in_maps = []
    for c in range(8):
        b, g = c // 4, c % 4
        in_maps.append(
            {
                "xT": np.ascontiguousarray(x[b].T).astype(np.float16),
                "wq": _pack_pm(wq[:, QW * g : QW * (g + 1)]),
                "wk": _pack_pm(wk[:, P * g : P * (g + 1)]),
                "wv": _pack_pm(wv[:, P * g : P * (g + 1)]),
                "wo": _pack_pm(wo[QW * g : QW * (g + 1), :]),
            }
        )
    return in_maps


def kernel(x, wq, wk, wv, wo):
    nc = _get_nc()
    in_maps = make_in_maps(x, wq, wk, wv, wo)
    res = run_bass_kernel_spmd(nc, in_maps, list(range(8)))
    parts = [res.results[c]["outT"].astype(np.float32) for c in range(8)]
    out = np.stack(
        [
            (parts[0] + parts[1] + parts[2] + parts[3]).T,
            (parts[4] + parts[5] + parts[6] + parts[7]).T,
        ]
    ).astype(np.float32)
    return out


# revision 29
# speedup vs baseline: 1.0169x; 1.0169x over previous
"""Causal GQA self-attention with RoPE for TRN2, 8 NeuronCores.

Problem: B=2, S=2048, D=2048, H=16 q-heads, KV=4 kv-heads, HD=128.

Sharding: core c = (batch b = c//4, kv-group g = c%4). Each core computes
q-heads 4g..4g+3 and kv-head g for batch b in the transposed (S^T) layout,
then a partial output projection; host sums the 4 partials per batch.

v6 notes (on top of v4's fused PE-saturated design):
  - startup: x chunk 0 leads the sync DMA ring, wk/wv are halved and the
    first x chunks split so the first K/V matmul has data ~4us earlier;
    cos/sin/wq/masks/wo are dispatched strictly after the x stream they
    used to compete with.
  - PE warm-up: a few zero matmuls on a memset tile run during the DMA
    wait so the HAM clock-gate (1.2->2.4GHz after ~3.4us busy) lifts
    before the real stream begins.
  - the ops accumulator, softmax-denominator psum and output-projection
    psum share one 4-slot PSUM ring: a head's first PV no longer waits
    on the previous head's attnT write (was ~0.7us/head).
  - diagonal exp tiles are narrowed to the live query range (ACT slack).
  - drained output-projection matmuls are emitted BEFORE each pair's PV
    (the PE queue is strict FIFO: behind PV they could not fill the
    exp-latency hole at each head start, ~1us/head); the per-head tail
    drains likewise moved ahead of the colsum/recip chain.
  - job-drain cadence slowed (odd pairs only for jq2/jq3) so output-
    projection fill lasts into the late heads; the final 16 jobs store
    2 chunks per DMA descriptor.
"""
import sys

sys.path.insert(0, "/opt/trn_rl_repo")

import numpy as np

import concourse.tile as tile
from concourse import bacc, mybir
from concourse.bass_utils import run_bass_kernel_spmd

F32 = mybir.dt.float32
F16 = mybir.dt.float16
AF = mybir.ActivationFunctionType
OP = mybir.AluOpType

P = 128          # partitions / head dim
S = 2048         # sequence length
D = 2048         # model dim
NH = 4           # q heads per core
QW = NH * P      # q projection width per core (512)
NKD = D // P     # contraction chunks (16)
QCH = 512        # query chunk (free dim of attention matmuls)
NQC = S // QCH   # 4
KCH = P          # key chunk (128, on partitions)
NKC = S // KCH   # 16
NOC = D // P     # output-projection chunks (16)
SCALE = float(P) ** -0.5


def _host_constants():
    inv = 1.0 / (10000.0 ** (np.arange(0, P, 2, dtype=np.float64) / P))  # [64]
    pos = np.arange(S, dtype=np.float64)
    freqs = pos[:, None] * inv[None, :]                  # [S, 64]
    emb = np.concatenate([freqs, freqs], axis=-1)        # [S, 128]
    cosT = np.cos(emb).T.astype(np.float16).copy()       # [128, S]
    sinT = np.sin(emb).T.astype(np.float16)
    sinT[: P // 2] *= np.float16(-1.0)                   # fold rotate_half sign
    sinT = sinT.copy()
    # triangular mask for the 128-wide diagonal sliver: m[p, q] = q >= p
    tri = (np.arange(P)[None, :] >= np.arange(P)[:, None]).astype(np.float16)
    # full step masks for the jq0/h0 full-width path: mask[p,j,q] = q >= p+128j
    q = np.arange(QCH)[None, None, :]
    p = np.arange(P)[:, None, None]
    j = np.arange(4)[None, :, None]
    masks = (q >= p + KCH * j).astype(np.float16)        # [128, 4, 512]
    ones = np.ones((P, P), dtype=np.float16)
    # rotate-half permutation: out[m] = in[(m+64) % 128]
    perm = np.zeros((P, P), dtype=np.float16)
    perm[(np.arange(P) + P // 2) % P, np.arange(P)] = 1.0
    return cosT, sinT, tri, masks, ones, perm


def build_nc():
    cosT_np, sinT_np, tri_np, masks_np, ones_np, perm_np = _host_constants()

    nc = bacc.Bacc(None)
    # weights arrive pre-packed as [128, ...] partition-major arrays
    xT_d = nc.dram_tensor("xT", [D, S], F16, kind="ExternalInput")
    wq_d = nc.dram_tensor("wq", [P, NKD * QW], F16, kind="ExternalInput")
    wk_d = nc.dram_tensor("wk", [P, NKD * P], F16, kind="ExternalInput")
    wv_d = nc.dram_tensor("wv", [P, NKD * P], F16, kind="ExternalInput")
    wo_d = nc.dram_tensor("wo", [P, NH * D], F16, kind="ExternalInput")
    out_d = nc.dram_tensor("outT", [D, S], F16, kind="ExternalOutput")

    cos_d = nc.inline_tensor(cosT_np, name="cosT")
    sin_d = nc.inline_tensor(sinT_np, name="sinT")
    perm_d = nc.inline_tensor(perm_np, name="permm")
    del tri_np, masks_np, ones_np  # generated on-device (DMA diet)

    xT_v = xT_d[:].rearrange("(kd p) s -> p kd s", p=P)
    out_v = out_d[:].rearrange("(oc p) s -> p oc s", p=P)

    # alternate DMA dispatch between the sync and gpsimd queues
    dq = [0]

    def dma(out, in_):
        eng = nc.sync if dq[0] % 2 == 0 else nc.gpsimd
        dq[0] += 1
        eng.dma_start(out, in_)

    with tile.TileContext(nc) as tc:
        with tc.tile_pool(name="persist", bufs=1) as pp:
            qT = pp.tile([P, NH, S], F16)        # q^T; attention overwrites
            kT = pp.tile([P, S], F16)
            vT = pp.tile([P, S], F16)
            vK = pp.tile([P, NKC, P], F16)       # V as (kpos, kchunk, hd)
            mask_t = pp.tile([P, 2, QCH], F16)   # step mask, on-device gen
            ones_t = pp.tile([P, P], F16)        # all-ones for colsum
            perm_t = pp.tile([P, P], F16)
            # triangular [P,P] sliver mask is the j=0 prefix of mask_t
            tri_t = mask_t[:, 0, 0:P]

            # attention output overwrites qT in place: slice (h, jq-chunk) is
            # written only after every read of that same slice is done.
            attnT = qT

            # ======== Phase 1: QKV projections + RoPE ====================
            # xp/wp/p1/p1c stay open through the fused phase: the q3 head
            # projection is deferred into the first attention block
            # (which otherwise idles PE waiting on exp latency)
            _outer = [
                tc.tile_pool(name="xp", bufs=1),
                tc.tile_pool(name="wp", bufs=1),
                tc.tile_pool(name="p1", bufs=2),
                tc.tile_pool(name="p1c", bufs=1),
            ]
            xp = _outer[0].__enter__()
            wp = _outer[1].__enter__()
            p1 = _outer[2].__enter__()
            p1c = _outer[3].__enter__()
            with tc.tile_pool(name="psP", bufs=1, space="PSUM") as psP:
                wkt = wp.tile([P, NKD, P], F16)
                wvt = wp.tile([P, NKD, P], F16)
                wqt = wp.tile([P, NKD, QW], F16)
                xf = xp.tile([P, NKD, S], F16)
                cos_t = p1c.tile([P, S], F16)
                sin_t = p1c.tile([P, S], F16)
                wk_v = wk_d[:].rearrange("p (kd c) -> p kd c", c=P)
                wv_v = wv_d[:].rearrange("p (kd c) -> p kd c", c=P)
                wq_vv = wq_d[:].rearrange("p (kd c) -> p kd c", c=QW)
                H8 = NKD // 2

                # PE warm-up: lift the HAM clock gate during the x DMA wait
                warm_w = p1c.tile([P, P], F16)
                warm_x = p1c.tile([P, QCH], F16)
                nc.vector.memset(warm_w[:], 0.0)
                nc.vector.memset(warm_x[:], 0.0)
                warm_ps = psP.tile([P, QCH], F32, tag="pk0", name="pk0")
                for _ in range(8):
                    nc.tensor.matmul(warm_ps[:], warm_w[:], warm_x[:],
                                     start=True, stop=True)

                # dispatch order == need order; the x stream round-robins
                # over FOUR queues (each DMA ring tops out well below the
                # per-core HBM budget, so more rings saturate sooner);
                # weights/constants follow strictly behind the x stream
                HS = S // 2
                sync_loads = [
                    (xf[:, 0, :HS], xT_v[:, 0, :HS]),
                    (xf[:, 0, HS:], xT_v[:, 0, HS:]),
                    (wkt[:, :H8, :], wk_v[:, :H8, :]),
                    (xf[:, 2, :HS], xT_v[:, 2, :HS]),
                    (xf[:, 2, HS:], xT_v[:, 2, HS:]),
                    (xf[:, 4, :], xT_v[:, 4, :]),
                    (xf[:, 6, :], xT_v[:, 6, :]),
                    (xf[:, 8, :], xT_v[:, 8, :]),
                    (xf[:, 10, :], xT_v[:, 10, :]),
                    (xf[:, 12, :], xT_v[:, 12, :]),
                    (xf[:, 14, :], xT_v[:, 14, :]),
                    (cos_t[:], cos_d[:]),
                    (wqt[:, 0:4, :], wq_vv[:, 0:4, :]),
                    (wqt[:, 8:12, :], wq_vv[:, 8:12, :]),
                ]
                gp_loads = [
                    (wvt[:, :H8, :], wv_v[:, :H8, :]),
                    (xf[:, 1, :HS], xT_v[:, 1, :HS]),
                    (xf[:, 1, HS:], xT_v[:, 1, HS:]),
                    (wkt[:, H8:, :], wk_v[:, H8:, :]),
                    (wvt[:, H8:, :], wv_v[:, H8:, :]),
                    (xf[:, 3, :HS], xT_v[:, 3, :HS]),
                    (xf[:, 3, HS:], xT_v[:, 3, HS:]),
                    (xf[:, 5, :], xT_v[:, 5, :]),
                    (xf[:, 7, :], xT_v[:, 7, :]),
                    (xf[:, 9, :], xT_v[:, 9, :]),
                    (xf[:, 11, :], xT_v[:, 11, :]),
                    (xf[:, 13, :], xT_v[:, 13, :]),
                    (xf[:, 15, :], xT_v[:, 15, :]),
                    (perm_t[:], perm_d[:]),
                    (sin_t[:], sin_d[:]),
                    (wqt[:, 4:8, :], wq_vv[:, 4:8, :]),
                    (wqt[:, 12:16, :], wq_vv[:, 12:16, :]),
                ]
                for o, i_ in sync_loads:
                    nc.sync.dma_start(o, i_)
                for o, i_ in gp_loads:
                    nc.gpsimd.dma_start(o, i_)

                # on-device constants (DMA diet): the causal step mask
                # (tri is its j=0 prefix) and the all-ones colsum matrix
                nc.gpsimd.memset(ones_t[:], 1.0)
                nc.gpsimd.memset(mask_t[:], 1.0)
                nc.gpsimd.affine_select(
                    mask_t[:], mask_t[:],
                    pattern=[[-KCH, 2], [1, QCH]],
                    compare_op=OP.is_ge, fill=0.0,
                    base=0, channel_multiplier=-1,
                )

                def rope_chunk(dst, rot_ps, cslice):
                    # dst = dst*cos + perm(dst)*sin ; rot_ps holds perm(dst)
                    tmpv = p1.tile([P, QCH], F16, tag="ropet", bufs=3)
                    nc.vector.tensor_tensor(
                        tmpv[:], rot_ps[:], sin_t[:, cslice], OP.mult)
                    nc.vector.tensor_tensor(
                        dst, dst, cos_t[:, cslice], OP.mult)
                    nc.vector.tensor_tensor(dst, dst, tmpv[:], OP.add)

                # K and V projections, kd-outer, 8 PSUM accumulators
                psK = [psP.tile([P, QCH], F32, tag=f"pk{jr}", name=f"pk{jr}")
                       for jr in range(NQC)]
                psV = [psP.tile([P, QCH], F32, tag=f"pv{jr}", name=f"pv{jr}")
                       for jr in range(NQC)]
                for kd in range(NKD):
                    for jr in range(NQC):
                        nc.tensor.matmul(
                            psK[jr][:], wkt[:, kd, :],
                            xf[:, kd, jr * QCH : (jr + 1) * QCH],
                            start=(kd == 0), stop=(kd == NKD - 1),
                        )
                    for jr in range(NQC):
                        nc.tensor.matmul(
                            psV[jr][:], wvt[:, kd, :],
                            xf[:, kd, jr * QCH : (jr + 1) * QCH],
                            start=(kd == 0), stop=(kd == NKD - 1),
                        )
                for jr in range(NQC):
                    nc.scalar.copy(
                        out=kT[:, jr * QCH : (jr + 1) * QCH], in_=psK[jr][:]
                    )
                # pre-warm the exp table set while ACT is idle-ish
                warm = p1.tile([P, 1], F32, tag="warm")
                nc.scalar.activation(warm[:], psK[0][:, 0:1], AF.Exp, scale=1.0)
                for jr in range(NQC):
                    nc.scalar.copy(
                        out=vT[:, jr * QCH : (jr + 1) * QCH], in_=psV[jr][:]
                    )
                # rope kT chunk by chunk (perm matmul reuses freed V banks)
                for jr in range(NQC):
                    cs = slice(jr * QCH, (jr + 1) * QCH)
                    rot = psP.tile([P, QCH], F32, tag=f"pv{jr}",
                                   name=f"pv{jr}")
                    nc.tensor.matmul(rot[:], perm_t[:], kT[:, cs],
                                     start=True, stop=True)
                    rope_chunk(kT[:, cs], rot, cs)
                nc.sync.dma_start_transpose(vK[:], vT[:])

                # Q projections per head, ping-pong PSUM; each chunk's
                # rot/rope is deferred by one chunk so the rot matmul
                # (which waits on the ACT copy) never blocks the PE FIFO
                pending_rope = [None]

                def flush_rope(idx):
                    if pending_rope[0] is None:
                        return
                    pdst, pcs = pending_rope[0]
                    pending_rope[0] = None
                    rot = psP.tile([P, QCH], F32, tag=f"pk{2 + idx % 2}",
                                   name=f"pk{2 + idx % 2}")
                    nc.tensor.matmul(rot[:], perm_t[:], pdst,
                                     start=True, stop=True)
                    rope_chunk(pdst, rot, pcs)

                ci = 0
                for hh in range(NH - 1):
                    for jr in range(NQC):
                        cs = slice(jr * QCH, (jr + 1) * QCH)
                        ps = psP.tile([P, QCH], F32, tag=f"pk{jr % 2}",
                                      name=f"pk{jr % 2}")
                        for kd in range(NKD):
                            nc.tensor.matmul(
                                ps[:],
                                wqt[:, kd, hh * P : (hh + 1) * P],
                                xf[:, kd, jr * QCH : (jr + 1) * QCH],
                                start=(kd == 0), stop=(kd == NKD - 1),
                            )
                        dst = qT[:, hh, cs]
                        nc.scalar.copy(out=dst, in_=ps[:])
                        flush_rope(ci)
                        pending_rope[0] = (dst, cs)
                        ci += 1
                flush_rope(ci)

            # wo prefetched during phase 1 tail / attention start
            p3w_cm = tc.tile_pool(name="p3w", bufs=1)
            p3w = p3w_cm.__enter__()
            wo_t = p3w.tile([P, NH, D], F16)
            wo_vv = wo_d[:].rearrange("p (a o) -> p a o", a=NH)
            dma(wo_t[:, 0:2, :], wo_vv[:, 0:2, :])
            dma(wo_t[:, 2:4, :], wo_vv[:, 2:4, :])

            # ======== Phase 2: fused attention + output projection =======
            # PSUM budget (8 banks): s0,s1 = 2+2; ops/dps/po share a 4-slot
            # ring (so a head's ops never waits on the previous head's
            # attnT write, and the output-projection pipeline is 3 deep).
            with tc.tile_pool(name="p2", bufs=1) as p2, \
                 tc.tile_pool(name="psF", bufs=1, space="PSUM") as psF:

                def ring():
                    return psF.tile([P, QCH], F32, tag="po", bufs=4,
                                    name="po")

                # pending output-projection emissions, drained into the
                # exp-latency gaps of the attention stream
                jobs = []

                # deferred q3 projection: 4-matmul groups drained into
                # jq1's h0/h1 exp-latency gaps via the shared ring;
                # q3 is first read at jq1's h3
                q3slot = [None]

                def make_qgroup(jr, g):
                    def qg():
                        cs = slice(jr * QCH, (jr + 1) * QCH)
                        dst = qT[:, 3, cs]
                        if g == 4:
                            # separate drain step: the rot matmul waits on
                            # the g3 ACT copy, so it must not ride the PE
                            # FIFO right behind it
                            rot = ring()
                            nc.tensor.matmul(rot[:], perm_t[:], dst,
                                             start=True, stop=True)
                            rope_chunk(dst, rot, cs)
                            return
                        if g == 0:
                            q3slot[0] = ring()
                        ps = q3slot[0]
                        for kd in range(4 * g, 4 * g + 4):
                            nc.tensor.matmul(
                                ps[:],
                                wqt[:, kd, 3 * P : 4 * P],
                                xf[:, kd, jr * QCH : (jr + 1) * QCH],
                                start=(kd == 0), stop=(kd == NKD - 1),
                            )
                        if g == 3:
                            nc.scalar.copy(out=dst, in_=ps[:])
                    return qg

                qjobs = [make_qgroup(jr, g)
                         for jr in range(NQC) for g in range(5)]

                def qdrain(n):
                    for _ in range(n):
                        if qjobs:
                            qjobs.pop(0)()

                quota = [1 << 30]

                def drain(n):
                    for _ in range(n):
                        if not jobs or quota[0] <= 0:
                            return
                        quota[0] -= 1
                        jobs.pop(0)()

                def make_job(oc, jq):
                    # full 4-head output projection chunk (jq0/jq2)
                    def job():
                        po = ring()
                        for a in range(NH):
                            nc.tensor.matmul(
                                po[:],
                                wo_t[:, a, oc * P : (oc + 1) * P],
                                attnT[:, a, jq * QCH : (jq + 1) * QCH],
                                start=(a == 0), stop=(a == NH - 1),
                            )
                        ot = p2.tile([P, QCH], F16, tag="ot", bufs=4)
                        if oc % 2 == 0:
                            nc.scalar.copy(out=ot[:], in_=po[:])
                        else:
                            nc.vector.tensor_copy(out=ot[:], in_=po[:])
                        dma(out_d[oc * P : (oc + 1) * P,
                                  jq * QCH : (jq + 1) * QCH], ot[:])
                    return job

                def make_final_pair(oc0, jq):
                    # two full chunks sharing one store descriptor (the
                    # final drain, where store-dispatch backlog matters)
                    def job():
                        ot2 = p2.tile([P, 2, QCH], F16, tag="ot2", bufs=3,
                                      name="ot2")
                        for k in range(2):
                            oc = oc0 + k
                            po = ring()
                            for a in range(NH):
                                nc.tensor.matmul(
                                    po[:],
                                    wo_t[:, a, oc * P : (oc + 1) * P],
                                    attnT[:, a, jq * QCH : (jq + 1) * QCH],
                                    start=(a == 0), stop=(a == NH - 1),
                                )
                            if k == 0:
                                nc.scalar.copy(out=ot2[:, k, :], in_=po[:])
                            else:
                                nc.vector.tensor_copy(out=ot2[:, k, :],
                                                      in_=po[:])
                        dma(out_v[:, oc0 : oc0 + 2,
                                  jq * QCH : (jq + 1) * QCH], ot2[:])
                    return job

                # jq1 first: its leading pairs are off-diagonal, so the
                # s tiles' first-ever writes are full-width (no stale
                # columns ever feed downstream), and by the time the
                # tiny DVE-bound jq0 block runs there are output-
                # projection jobs available to keep PE busy.
                for jq in (1, 0, 2, 3):
                    for h in range(NH):
                        nkc = 4 * (jq + 1)
                        npair = nkc // 2
                        qs = qT[:, h, jq * QCH : (jq + 1) * QCH]
                        if jq == 1 and h == 2:
                            # q3 must be fully projected+roped before h3
                            qdrain(len(qjobs))
                        # ration the job fill evenly over this jq's heads
                        quota[0] = max(1, -(-len(jobs) // (NH - h)))
                        ops = ring()
                        pAcc = p2.tile([P, 2 * QCH], F16,
                                       tag="pAcc", bufs=2, name="pAcc")

                        # query offset of the live range for key chunk kc
                        # (0 off the diagonal)
                        def qoff(kc):
                            return max(0, KCH * (kc - 4 * jq))

                        def emit_qk(ip):
                            kc0 = 2 * ip
                            sps = psF.tile(
                                [P, 2 * QCH], F32, tag=f"s{ip % 2}",
                                name=f"sps{ip % 2}",
                            )
                            for k2 in range(2):
                                off = qoff(kc0 + k2)
                                nc.tensor.matmul(
                                    sps[:, k2 * QCH + off : (k2 + 1) * QCH],
                                    kT[:, (kc0 + k2) * P : (kc0 + k2 + 1) * P],
                                    qs[:, off:],
                                    start=True,
                                    stop=True,
                                )
                            return sps

                        sps_cur = emit_qk(0)
                        for ip in range(npair):
                            kc0 = 2 * ip
                            pT = p2.tile([P, 2 * QCH], F16, tag="pT", bufs=6)
                            diag = kc0 >= 4 * jq
                            first_pair_init = ip == 0
                            if (diag and not first_pair_init
                                    and qoff(kc0 + 1) > 0):
                                # narrow exp to the live query range
                                for k2 in range(2):
                                    off = qoff(kc0 + k2)
                                    sl = slice(k2 * QCH + off,
                                               (k2 + 1) * QCH)
                                    nc.scalar.activation(
                                        pT[:, sl], sps_cur[:, sl],
                                        AF.Exp, scale=SCALE
                                    )
                            else:
                                nc.scalar.activation(
                                    pT[:], sps_cur[:], AF.Exp, scale=SCALE
                                )
                            if diag and first_pair_init:
                                # jq0 pair0: full-width mask (also zeroes
                                # dead/stale columns) since the masked tile
                                # is about to initialize pAcc via a full-
                                # width copy
                                nc.vector.tensor_tensor(
                                    pT[:], pT[:],
                                    mask_t[:, kc0 : kc0 + 2, :], OP.mult,
                                )
                            elif diag:
                                # mask only the 128-wide diagonal slivers
                                for k2 in range(2):
                                    off = qoff(kc0 + k2)
                                    sl = slice(k2 * QCH + off,
                                               k2 * QCH + off + KCH)
                                    nc.vector.tensor_tensor(
                                        pT[:, sl], pT[:, sl], tri_t[:],
                                        OP.mult,
                                    )
                            if ip + 1 < npair:
                                sps_cur = emit_qk(ip + 1)
                            # fill the exp-latency hole BEFORE the PV that
                            # waits on it (PE queue is strict FIFO)
                            if jq == 1 and qjobs:
                                qdrain(1)
                            elif jq == 0:
                                drain(2)
                            elif ip == 0 or ip % 2 == 1:
                                # jq2/jq3: pair 0 (the head-start exp hole)
                                # and odd pairs; the per-head quota spreads
                                # the 16 jobs over all four heads
                                drain(1)
                            for k2 in range(2):
                                kc = kc0 + k2
                                off = qoff(kc)
                                nc.tensor.matmul(
                                    ops[:, off:],
                                    vK[:, kc, :],
                                    pT[:, k2 * QCH + off : (k2 + 1) * QCH],
                                    start=(kc == 0),
                                    stop=(kc == nkc - 1),
                                )
                            if first_pair_init:
                                nc.vector.tensor_copy(out=pAcc[:], in_=pT[:])
                            elif qoff(kc0) == 0 and qoff(kc0 + 1) == 0:
                                nc.vector.tensor_tensor(
                                    pAcc[:], pAcc[:], pT[:], OP.add
                                )
                            else:
                                for k2 in range(2):
                                    off = qoff(kc0 + k2)
                                    sl = slice(k2 * QCH + off, (k2 + 1) * QCH)
                                    nc.vector.tensor_tensor(
                                        pAcc[:, sl], pAcc[:, sl], pT[:, sl],
                                        OP.add,
                                    )
                        def tail(ops=ops, pAcc=pAcc, h=h, jq=jq):
                            # PE runway first: the colsum waits on the DVE
                            # pAcc chain anyway, and the next head's QK can
                            # run during the recip/mult latency
                            if jobs:
                                drain(3)
                            else:
                                qdrain(3)
                            # cross-partition colsum of pAcc -> denominator
                            dps = ring()
                            if jq >= 2:
                                # DVE has slack here: pre-add the halves
                                # so one colsum matmul suffices
                                pAccH = p2.tile([P, QCH], F16, tag="pAccH",
                                                bufs=2, name="pAccH")
                                nc.vector.tensor_tensor(
                                    pAccH[:], pAcc[:, 0:QCH],
                                    pAcc[:, QCH : 2 * QCH], OP.add,
                                )
                                nc.tensor.matmul(
                                    dps[:], ones_t[:], pAccH[:],
                                    start=True, stop=True,
                                )
                            else:
                                nc.tensor.matmul(
                                    dps[:], ones_t[:], pAcc[:, 0:QCH],
                                    start=True, stop=False,
                                )
                                nc.tensor.matmul(
                                    dps[:], ones_t[:],
                                    pAcc[:, QCH : 2 * QCH],
                                    start=False, stop=True,
                                )
                            dib = p2.tile([P, QCH], F32, tag="dib", bufs=2,
                                          name="dib")
                            nc.vector.reciprocal_approx_fast(dib[:], dps[:])
                            nc.vector.tensor_tensor(
                                attnT[:, h, jq * QCH : (jq + 1) * QCH],
                                ops[:],
                                dib[:],
                                OP.mult,
                            )

                        tail()
                    if jq == 3:
                        for oc0 in range(0, NOC, 2):
                            jobs.append(make_final_pair(oc0, jq))
                    else:
                        for oc in range(NOC):
                            jobs.append(make_job(oc, jq))
                quota[0] = 1 << 30
                drain(len(jobs))
            p3w_cm.__exit__(None, None, None)
            for _cm in reversed(_outer):
                _cm.__exit__(None, None, None)

    nc.finalize()
    return nc


_NC = None


def _get_nc():
    global _NC
    if _NC is None:
        _NC = build_nc()
    return _NC


def _pack_pm(w):
    """[K, C] f32 -> [128, (K//128)*C] f16 partition-major pack:
    out[p, kd*C + c] = w[kd*128 + p, c]"""
    K, C = w.shape
    kd = K // P
    return np.ascontiguousarray(
        np.asarray(w, dtype=np.float16).reshape(kd, P, C).transpose(1, 0, 2)
    ).reshape(P, kd * C)


def make_in_maps(x, wq, wk, wv, wo):
    x = np.asarray(x, dtype=np.float32)
    in_maps = []
    for c in range(8):
        b, g = c // 4, c % 4
        in_maps.append(
            {
                "xT": np.ascontiguousarray(x[b].T).astype(np.float16),
                "wq": _pack_pm(wq[:, QW * g : QW * (g + 1)]),
                "wk": _pack_pm(wk[:, P * g : P * (g + 1)]),
                "wv": _pack_pm(wv[:, P * g : P * (g + 1)]),
                "wo": _pack_pm(wo[QW * g : QW * (g + 1), :]),
            }
        )
    return in_maps


def kernel(x, wq, wk, wv, wo):
    nc = _get_nc()
    in_maps = make_in_maps(x, wq, wk, wv, wo)
    res = run_bass_kernel_spmd(nc, in_maps, list(range(8)))
    parts = [res.results[c]["outT"].astype(np.float32) for c in range(8)]
    out = np.stack(
        [
            (parts[0] + parts[1] + parts[2] + parts[3]).T,
            (parts[4] + parts[5] + parts[6] + parts[7]).T,
        ]
    ).astype(np.float32)
    return out
